# revision 13
# baseline (speedup 1.0000x reference)
"""MeanStdMax pooling kernel for Trainium2 (8 NeuronCores, data-parallel).

Input : hidden_states [16, 13, 512, 768] fp32
Output: [16, 13, 2304] fp32 = concat([sum(seq), std(seq, ddof=1), max(seq)], -1)

Sharding: batch dim 16 -> 2 batches per core (no cross-core communication).

Per-core plan (26 (b,l) pairs, each [512, 768]):
  - DMA pair as one [128, 4*768] SBUF tile (partition = seq%128, contiguous rows)
  - max : DVE tensor_tensor(max) tree over the 4 seq blocks -> [128,768],
          then GPSIMD partition_all_reduce(max) -> per-pair [768] row
  - sum : GPSIMD+DVE adds over seq blocks -> [128,768], then fp32r matmul with a
          one-hot [128,32] weight column j so PSUM row j accumulates pair j's sums
  - sumsq: ACT Square -> [128, 4*768], fp32r matmuls into second PSUM accumulator
  - epilogue: std = sqrt((sumsq - sum^2/512)/511) computed batched on [26,768]
"""

import os
import sys

import numpy as np

for _p in ("/opt/trn_rl_repo", "/root/.axon_site/_ro/trn_rl_repo"):
    if os.path.isdir(_p) and _p not in sys.path:
        sys.path.insert(0, _p)

import concourse.bacc as bacc
import concourse.bass as bass
import concourse.bass_isa as bass_isa
import concourse.mybir as mybir
import concourse.tile as tile
from concourse.bass_utils import run_bass_kernel_spmd

N_CORES = 8
B_FULL, L, S, H = 16, 13, 512, 768
B = B_FULL // N_CORES  # 2 batches per core
P = 128
NBLK = S // P  # 4
NPAIR = B * L  # 26
F32 = mybir.dt.float32
F32R = mybir.dt.float32r

_CACHE = {}


def _build():
    if "nc" in _CACHE:
        return _CACHE["nc"]

    nc = bacc.Bacc("TRN2", target_bir_lowering=False, debug=False,
                   num_devices=N_CORES)
    # float32r: same bits as fp32, but satisfies the BIR verifier's
    # "rounded to FP32r" rule so DMA-loaded tiles can feed fp32r matmuls
    # (the fast single-pass fp32 PE mode, ~0.5ns/row vs 1.7 for fp32).
    x = nc.dram_tensor("x", [B, L, S, H], F32R, kind="ExternalInput").ap()
    out = nc.dram_tensor("out", [B, L, 3 * H], F32, kind="ExternalOutput").ap()
    out2 = out.rearrange("b l h -> (b l) h")  # [26, 2304]

    BF16 = mybir.dt.bfloat16
    with tile.TileContext(nc) as tc:
        with (
            tc.tile_pool(name="inp", bufs=4) as in_pool,
            tc.tile_pool(name="sq", bufs=3) as sq_pool,
            tc.tile_pool(name="acc", bufs=3) as acc_pool,
            tc.tile_pool(name="red", bufs=3) as red_pool,
            tc.tile_pool(name="const", bufs=1) as const_pool,
            tc.tile_pool(name="ep", bufs=1) as ep_pool,
            tc.tile_pool(name="psum", bufs=1, space="PSUM") as psum_pool,
        ):
            # one-hot weight bank: W[:, 26-j : 58-j] is all-ones exactly at
            # local column j.
            W0 = const_pool.tile([P, NPAIR + 32], F32)
            nc.gpsimd.memset(W0[:], 0.0)
            nc.gpsimd.memset(W0[:, NPAIR:NPAIR + 1], 1.0)
            Wr = const_pool.tile([P, NPAIR + 32], F32R)
            nc.vector.tensor_copy(Wr[:], W0[:])
            Wb = const_pool.tile([P, NPAIR + 32], BF16)
            nc.vector.tensor_copy(Wb[:], W0[:])

            ps_sum_a = psum_pool.tile([32, 512], F32)
            ps_sum_b = psum_pool.tile([32, 256], F32)
            ps_sq_a = psum_pool.tile([32, 512], F32)
            ps_sq_b = psum_pool.tile([32, 256], F32)

            # PE is software-pipelined one pair behind DVE for the sq matmuls
            pending_sq = None  # (j, qf_tile)

            def emit_sq_matmuls(j, qf):
                first, last = (j == 0), (j == NPAIR - 1)
                wjb = Wb[:, NPAIR - j:NPAIR - j + 32]
                nc.tensor.matmul(ps_sq_a[:], wjb, qf[:, 0:512],
                                 start=first, stop=last)
                nc.tensor.matmul(ps_sq_b[:], wjb, qf[:, 512:768],
                                 start=first, stop=last)

            for j in range(NPAIR):
                b, l = divmod(j, L)
                first, last = (j == 0), (j == NPAIR - 1)

                T = in_pool.tile([P, NBLK * H], F32R)
                Tr = T[:].rearrange("p (n h) -> p n h", h=H)
                # partition p holds seq rows 4p..4p+3 -> each partition's row
                # is one fully contiguous 12KB DRAM chunk (best DMA shape);
                # the seq->(p,i) mapping is irrelevant to sum/max/sumsq.
                nc.sync.dma_start(
                    T[:], x[b, l].rearrange("(p n) h -> p n h", n=NBLK))
                Tv = T[:].bitcast(F32).rearrange("p (n h) -> p n h", h=H)

                # ---- max over seq: DVE tree + gpsimd partition all-reduce ----
                m2 = acc_pool.tile([P, 2 * H], F32, tag="m2")
                m2v = m2[:].rearrange("p (n h) -> p n h", h=H)
                nc.vector.tensor_tensor(
                    m2v, Tv[:, 0:2, :], Tv[:, 2:4, :], op=mybir.AluOpType.max)
                M = acc_pool.tile([P, H], F32, tag="M")
                nc.vector.tensor_tensor(
                    M[:], m2v[:, 0, :], m2v[:, 1, :], op=mybir.AluOpType.max)
                Mred = red_pool.tile([P, H], F32, tag="Mred")
                nc.gpsimd.partition_all_reduce(
                    Mred[:], M[:], channels=P, reduce_op=bass_isa.ReduceOp.max)
                nc.sync.dma_start(out2[j:j + 1, 2 * H:3 * H], Mred[0:1, :])

                # ---- sums: fp32r one-hot matmuls straight off the raw tile ----
                wjr = Wr[:, NPAIR - j:NPAIR - j + 32]
                for blk in range(NBLK):
                    nc.tensor.matmul(
                        ps_sum_a[:], wjr, Tr[:, blk, 0:512],
                        start=first and blk == 0, stop=last and blk == NBLK - 1)
                    nc.tensor.matmul(
                        ps_sum_b[:], wjr, Tr[:, blk, 512:768],
                        start=first and blk == 0, stop=last and blk == NBLK - 1)

                # ---- squares in bf16 on ACT; 2x-mode accumulation on DVE ----
                Q = sq_pool.tile([P, NBLK * H], BF16)
                nc.scalar.activation(Q[:], T[:].bitcast(F32),
                                     mybir.ActivationFunctionType.Square)
                Qv = Q[:].rearrange("p (n h) -> p n h", h=H)
                q2 = acc_pool.tile([P, 2 * H], BF16, tag="q2")
                q2v = q2[:].rearrange("p (n h) -> p n h", h=H)
                qf = acc_pool.tile([P, H], BF16, tag="qf")
                with nc.allow_low_precision("bf16 sumsq partials"):
                    nc.vector.tensor_tensor(
                        q2v, Qv[:, 0:2, :], Qv[:, 2:4, :],
                        op=mybir.AluOpType.add)
                    nc.vector.tensor_tensor(
                        qf[:], q2v[:, 0, :], q2v[:, 1, :],
                        op=mybir.AluOpType.add)

                if pending_sq is not None:
                    emit_sq_matmuls(*pending_sq)
                pending_sq = (j, qf)

            emit_sq_matmuls(*pending_sq)

            # ---- epilogue: sums out + std = sqrt((sumsq - sum^2/n)/(n-1)) ----
            stats = ep_pool.tile([32, 2 * H], F32)
            nc.scalar.copy(stats[:, 0:512], ps_sum_a[:])
            nc.scalar.copy(stats[:, 512:768], ps_sum_b[:])

            sum2 = ep_pool.tile([32, H], F32)
            nc.vector.tensor_tensor(sum2[:], stats[:, 0:H], stats[:, 0:H],
                                    op=mybir.AluOpType.mult)
            nc.vector.tensor_scalar_mul(sum2[:], sum2[:], -1.0 / S)
            var = ep_pool.tile([32, H], F32)
            nc.vector.tensor_tensor(var[:, 0:512], ps_sq_a[:], sum2[:, 0:512],
                                    op=mybir.AluOpType.add)
            nc.vector.tensor_tensor(var[:, 512:768], ps_sq_b[:], sum2[:, 512:768],
                                    op=mybir.AluOpType.add)
            nc.scalar.activation(stats[:, H:2 * H], var[:],
                                 mybir.ActivationFunctionType.Sqrt,
                                 scale=1.0 / (S - 1))

            nc.sync.dma_start(out2[0:NPAIR, 0:2 * H], stats[0:NPAIR, :])

    nc.compile()
    _CACHE["nc"] = nc
    return nc


def _run(hidden_states: np.ndarray, trace: bool = False):
    nc = _build()
    x = np.ascontiguousarray(np.asarray(hidden_states, dtype=np.float32))
    assert x.shape == (B_FULL, L, S, H), x.shape
    in_maps = [{"x": x[c * B:(c + 1) * B]} for c in range(N_CORES)]
    res = run_bass_kernel_spmd(nc, in_maps, core_ids=list(range(N_CORES)),
                               trace=trace)
    out = np.empty((B_FULL, L, 3 * H), dtype=np.float32)
    for c in range(N_CORES):
        out[c * B:(c + 1) * B] = res.results[c]["out"]
    return out, res


def kernel(hidden_states: np.ndarray) -> np.ndarray:
    out, _ = _run(hidden_states)
    return out


# revision 17
# speedup vs baseline: 1.2069x; 1.2069x over previous
"""MeanStdMax pooling kernel for Trainium2 (8 NeuronCores, data-parallel).

Input : hidden_states [16, 13, 512, 768] fp32
Output: [16, 13, 2304] fp32 = concat([sum(seq), std(seq, ddof=1), max(seq)], -1)

Sharding: batch dim 16 -> 2 batches per core (no cross-core communication).

Per-core plan (26 (b,l) pairs, each [512, 768]):
  - DMA each pair as one [128, 4*768] tile; partition p holds seq rows
    4p..4p+3, so every partition is one contiguous 12KB DRAM chunk.
  - sum  : fp32r one-hot-weight matmuls straight off the raw tile; PSUM row j
           accumulates pair j's per-hidden sums (partition reduce on the PE).
  - sumsq: ACT Square -> bf16, then bf16 one-hot matmuls into 2nd accumulator.
  - max  : DVE max tree over the 4 seq blocks -> M [128,768]; PE-transpose M
           into PSUM; DVE reduce_max over the free axis -> maxout[:, 6j:6j+6];
           final PE transpose of maxout makes the output contiguous.
  - epilogue: std = sqrt((sumsq - sum^2/512)/511) batched over [26,768].
"""

import os
import sys

import numpy as np

for _p in ("/opt/trn_rl_repo", "/root/.axon_site/_ro/trn_rl_repo"):
    if os.path.isdir(_p) and _p not in sys.path:
        sys.path.insert(0, _p)

import concourse.bacc as bacc
import concourse.bass as bass
import concourse.masks as masks
import concourse.mybir as mybir
import concourse.tile as tile
from concourse.bass_utils import run_bass_kernel_spmd

N_CORES = 8
B_FULL, L, S, H = 16, 13, 512, 768
B = B_FULL // N_CORES  # 2 batches per core
P = 128
NBLK = S // P  # 4
NPAIR = B * L  # 26
NCH = H // P  # 6 hidden chunks of 128
F32 = mybir.dt.float32
F32R = mybir.dt.float32r
BF16 = mybir.dt.bfloat16

_CACHE = {}


def _build():
    if "nc" in _CACHE:
        return _CACHE["nc"]

    nc = bacc.Bacc("TRN2", target_bir_lowering=False, debug=False,
                   num_devices=N_CORES)
    # float32r: same bits as fp32, but satisfies the BIR verifier's
    # "rounded to FP32r" rule so DMA-loaded tiles can feed fp32r matmuls
    # (the fast single-pass fp32 PE mode, ~0.5ns/row vs 1.7 for fp32).
    x = nc.dram_tensor("x", [B, L, S, H], F32R, kind="ExternalInput").ap()
    out = nc.dram_tensor("out", [B, L, 3 * H], F32, kind="ExternalOutput").ap()
    out2 = out.rearrange("b l h -> (b l) h")  # [26, 2304]

    with tile.TileContext(nc) as tc:
        with (
            tc.tile_pool(name="inp", bufs=4) as in_pool,
            tc.tile_pool(name="sq", bufs=3) as sq_pool,
            tc.tile_pool(name="acc", bufs=3) as acc_pool,
            tc.tile_pool(name="const", bufs=1) as const_pool,
            tc.tile_pool(name="ep", bufs=1) as ep_pool,
            tc.tile_pool(name="psum", bufs=1, space="PSUM") as psum_pool,
            tc.tile_pool(name="psmx", bufs=2, space="PSUM") as psmx_pool,
        ):
            # one-hot weight bank: W[:, 26-j : 58-j] is all-ones exactly at
            # local column j.
            W0 = const_pool.tile([P, NPAIR + 32], F32)
            nc.gpsimd.memset(W0[:], 0.0)
            nc.gpsimd.memset(W0[:, NPAIR:NPAIR + 1], 1.0)
            Wr = const_pool.tile([P, NPAIR + 32], F32R)
            nc.vector.tensor_copy(Wr[:], W0[:])
            Wb = const_pool.tile([P, NPAIR + 32], BF16)
            nc.vector.tensor_copy(Wb[:], W0[:])
            ident = const_pool.tile([P, P], F32)
            masks.make_identity(nc, ident[:])

            ps_sum_a = psum_pool.tile([32, 512], F32)
            ps_sum_b = psum_pool.tile([32, 256], F32)
            ps_sq_a = psum_pool.tile([32, 512], F32)
            ps_sq_b = psum_pool.tile([32, 256], F32)

            # max results: maxout[p, 6j + c] = max over seq of pair j,
            # hidden index c*128 + p
            maxout = ep_pool.tile([P, NPAIR * NCH], F32)

            # PE runs one pair behind for sq matmuls / max transposes so its
            # per-iteration work only depends on data from iteration j-1.
            pending = None  # (j, Q_tile, M_tile)

            def emit_tail(j, Q, M):
                first, last = (j == 0), (j == NPAIR - 1)
                wjb = Wb[:, NPAIR - j:NPAIR - j + 32]
                Qv = Q[:].rearrange("p (n h) -> p n h", h=H)
                for blk in range(NBLK):
                    nc.tensor.matmul(
                        ps_sq_a[:], wjb, Qv[:, blk, 0:512],
                        start=first and blk == 0, stop=last and blk == NBLK - 1)
                    nc.tensor.matmul(
                        ps_sq_b[:], wjb, Qv[:, blk, 512:768],
                        start=first and blk == 0, stop=last and blk == NBLK - 1)
                # transpose M into PSUM, then reduce over seq-partition axis
                pmx = psmx_pool.tile([P, H], F32, tag="pmx")
                for c in range(NCH):
                    nc.tensor.transpose(
                        pmx[:, c * P:(c + 1) * P], M[:, c * P:(c + 1) * P],
                        ident[:])
                nc.vector.reduce_max(
                    maxout[:, j * NCH:(j + 1) * NCH],
                    pmx[:].rearrange("p (c s) -> p c s", s=P),
                    axis=mybir.AxisListType.X)
                # transpose the per-pair [128, 6] max column block to [6, 128]
                # and ship it out as one contiguous 3KB row
                pmxt = psmx_pool.tile([NCH, P], F32, tag="pmx")
                nc.tensor.transpose(
                    pmxt[:], maxout[:, j * NCH:(j + 1) * NCH], ident[:])
                mrow = acc_pool.tile([NCH, P], F32, tag="mrow")
                nc.scalar.copy(mrow[:], pmxt[:])
                nc.sync.dma_start(
                    out2[j, 2 * H:3 * H].rearrange("(c s) -> c s", s=P),
                    mrow[:])

            for j in range(NPAIR):
                b, l = divmod(j, L)
                first, last = (j == 0), (j == NPAIR - 1)

                T = in_pool.tile([P, NBLK * H], F32R)
                Tr = T[:].rearrange("p (n h) -> p n h", h=H)
                # partition p <- seq rows 4p..4p+3: contiguous 12KB chunks;
                # the seq->(p,i) mapping is irrelevant to sum/max/sumsq.
                nc.sync.dma_start(
                    T[:], x[b, l].rearrange("(p n) h -> p n h", n=NBLK))
                Tv = T[:].bitcast(F32).rearrange("p (n h) -> p n h", h=H)

                # ---- sums: fp32r one-hot matmuls straight off the raw tile ----
                wjr = Wr[:, NPAIR - j:NPAIR - j + 32]
                for blk in range(NBLK):
                    nc.tensor.matmul(
                        ps_sum_a[:], wjr, Tr[:, blk, 0:512],
                        start=first and blk == 0, stop=last and blk == NBLK - 1)
                    nc.tensor.matmul(
                        ps_sum_b[:], wjr, Tr[:, blk, 512:768],
                        start=first and blk == 0, stop=last and blk == NBLK - 1)

                # ---- max tree on DVE ----
                m2 = acc_pool.tile([P, 2 * H], F32, tag="m2")
                m2v = m2[:].rearrange("p (n h) -> p n h", h=H)
                nc.vector.tensor_tensor(
                    m2v, Tv[:, 0:2, :], Tv[:, 2:4, :], op=mybir.AluOpType.max)
                M = acc_pool.tile([P, H], F32, tag="M")
                nc.vector.tensor_tensor(
                    M[:], m2v[:, 0, :], m2v[:, 1, :], op=mybir.AluOpType.max)

                # ---- squares in bf16 on ACT ----
                Q = sq_pool.tile([P, NBLK * H], BF16)
                nc.scalar.activation(Q[:], T[:].bitcast(F32),
                                     mybir.ActivationFunctionType.Square)

                if pending is not None:
                    emit_tail(*pending)
                pending = (j, Q, M)

            emit_tail(*pending)

            # ---- epilogue: sums out + std = sqrt((sumsq - sum^2/n)/(n-1)) ----
            stats = ep_pool.tile([32, 2 * H], F32)
            nc.scalar.copy(stats[:, 0:512], ps_sum_a[:])
            nc.scalar.copy(stats[:, 512:768], ps_sum_b[:])

            sum2 = ep_pool.tile([32, H], F32)
            nc.vector.tensor_tensor(sum2[:], stats[:, 0:H], stats[:, 0:H],
                                    op=mybir.AluOpType.mult)
            nc.vector.tensor_scalar_mul(sum2[:], sum2[:], -1.0 / S)
            var = ep_pool.tile([32, H], F32)
            nc.vector.tensor_tensor(var[:, 0:512], ps_sq_a[:], sum2[:, 0:512],
                                    op=mybir.AluOpType.add)
            nc.vector.tensor_tensor(var[:, 512:768], ps_sq_b[:], sum2[:, 512:768],
                                    op=mybir.AluOpType.add)
            nc.scalar.activation(stats[:, H:2 * H], var[:],
                                 mybir.ActivationFunctionType.Sqrt,
                                 scale=1.0 / (S - 1))

            nc.sync.dma_start(out2[0:NPAIR, 0:2 * H], stats[0:NPAIR, :])

    nc.compile()
    _CACHE["nc"] = nc
    return nc


def _run(hidden_states: np.ndarray, trace: bool = False):
    nc = _build()
    x = np.ascontiguousarray(np.asarray(hidden_states, dtype=np.float32))
    assert x.shape == (B_FULL, L, S, H), x.shape
    in_maps = [{"x": x[c * B:(c + 1) * B]} for c in range(N_CORES)]
    res = run_bass_kernel_spmd(nc, in_maps, core_ids=list(range(N_CORES)),
                               trace=trace)
    out = np.empty((B_FULL, L, 3 * H), dtype=np.float32)
    for c in range(N_CORES):
        out[c * B:(c + 1) * B] = res.results[c]["out"]
    return out, res


def kernel(hidden_states: np.ndarray) -> np.ndarray:
    out, _ = _run(hidden_states)
    return out
